# revision 10
# baseline (speedup 1.0000x reference)
"""Trainium2 Bass kernel for nn_LocallyDense.

Computation (reference):
    xg[b,g,s] = x[b, idx[g,s]]                        # gather
    out[b,g,o] = sum_s xg[b,g,s] * W[g,s,o] + b[g,o]  # 360 grouped dense
    out = out * (gamma*rsqrt(var+eps)) + (beta - mean*gamma*rsqrt(var+eps))

Shapes: x [256, 65536] f32, idx [360, 128] i32, W [360,128,256] f32,
b [360,256], gamma/beta/mean/var [256].  Output [256, 360, 256] f32.

Strategy: shard the 360 groups over 8 cores (45 groups each; every core
keeps the full batch, so no collectives are needed — the host
concatenates the per-core outputs).  BN scale is folded into W on the
host, BN shift + b folded into a per-(group,out) bias.

The host transposes x to xT [65536, 256] (one voxel row = 1 KB
contiguous) and *compacts* it per core: each core only needs the <=5760
distinct voxel rows its 45 groups reference, so the host ships
xTc [5760, 256] plus remapped int16 indices.  The device gathers voxel
rows with the SWDGE `dma_gather` primitive (dst[i%128, i//128, :] =
src[idx[i], :]), which with i = g*128 + s yields exactly the transposed
activation tile xgT[s, g, b] needed for the grouped matmul.

Device per group g (o_half h in {0,1}):
    psum[128_o, 256_b] = W[g][:, h*128:+128].T @ xgT[:, g, :]  (TensorE)
    sbuf_out = psum + bias[g, h]     (ACT / DVE per-partition bias add)
    DMA out -> out_dev[h, o_local, g, b]  (layout gives k*1KB contiguous
                                           store descriptors)

Host epilogue: concatenate the 8 core outputs and transpose to [B,G,O].
"""

import numpy as np

import concourse.bass as bass
import concourse.bacc as bacc
import concourse.mybir as mybir
import concourse.tile as tile
from concourse.bass_utils import run_bass_kernel_spmd

# Problem constants (hardcoded per harness contract)
N_GROUPS, GROUP_SIZE, OUT_DIM = 360, 128, 256
N_VOXELS, BATCH = 65536, 256
BN_EPS = 1e-3
N_CORES = 8
G_PER = N_GROUPS // N_CORES        # 45 groups per core
O_HALVES = OUT_DIM // 128          # 2
N_ROWS = G_PER * GROUP_SIZE        # 5760 gathered rows per core

# Tuning knobs
# GB*128 indices per dma_gather call; with single_packet=True the per-lane
# packet is num_idxs/16+1 descriptors and must stay <= 64 -> GB <= 7.
GB = 5                             # groups per pipeline chunk
N_CHUNKS = G_PER // GB             # 9
assert N_CHUNKS * GB == G_PER
IDX_COLS = N_ROWS // 16            # 360 int16 per partition (wrap layout)
IDX_COLS_C = GB * GROUP_SIZE // 16  # 72 per chunk

F32 = mybir.dt.float32
I16 = mybir.dt.int16

_cached = {}


def build_kernel() -> bass.Bass:
    nc = bacc.Bacc("TRN2", target_bir_lowering=False, debug=False)
    # Inputs (per core)
    xTc = nc.dram_tensor("xTc", [N_ROWS, BATCH], F32, kind="ExternalInput")
    # Wd[s, g*256+o] = W_folded[g, s, o]
    Wd = nc.dram_tensor("Wd", [GROUP_SIZE, G_PER * OUT_DIM], F32, kind="ExternalInput")
    # idx16: wrap layout per gather chunk, replicated over the 8 Q7 cores
    idx16 = nc.dram_tensor("idx16", [128, IDX_COLS], I16, kind="ExternalInput")
    # biasd[p, h*G_PER+g] = bias[g, h*128+p]
    biasd = nc.dram_tensor("biasd", [128, O_HALVES * G_PER], F32, kind="ExternalInput")
    # Output: out_dev[h, o_local, g, b] = result[b, g, h*128+o_local]
    out = nc.dram_tensor(
        "out", [O_HALVES, 128, G_PER, BATCH], F32, kind="ExternalOutput"
    )

    with tile.TileContext(nc) as tc:
        with (
            tc.tile_pool(name="const", bufs=1) as cpool,
            tc.tile_pool(name="wpool", bufs=1) as wpool,
            tc.tile_pool(name="xpool", bufs=3) as xpool,
            tc.tile_pool(name="opool", bufs=4) as opool,
            tc.tile_pool(name="ppool", bufs=8, space="PSUM") as ppool,
        ):
            idx_t = cpool.tile([128, IDX_COLS], I16, name="idx_t")
            nc.sync.dma_start(out=idx_t[:], in_=idx16[:])
            bias_t = cpool.tile([128, O_HALVES * G_PER], F32, name="bias_t")
            nc.sync.dma_start(out=bias_t[:], in_=biasd[:])

            # Resident weight tiles, one per chunk; per-partition descriptors
            # are GB KB contiguous.
            w_tiles = []
            for c in range(N_CHUNKS):
                w_t = wpool.tile([GROUP_SIZE, GB * OUT_DIM], F32, name=f"w_{c}")
                nc.sync.dma_start(
                    out=w_t[:], in_=Wd[:, c * GB * OUT_DIM : (c + 1) * GB * OUT_DIM]
                )
                w_tiles.append(w_t)

            for c in range(N_CHUNKS):
                # Gather GB*128 voxel rows: xg[s, j, :] = xTc[cidx[(c*GB+j)*128+s], :]
                xg = xpool.tile([GROUP_SIZE, GB, BATCH], F32, name="xg")
                nc.gpsimd.dma_gather(
                    out_ap=xg[:],
                    in_ap=xTc[:],
                    idxs_ap=idx_t[:, c * IDX_COLS_C : (c + 1) * IDX_COLS_C],
                    num_idxs=GB * GROUP_SIZE,
                    num_idxs_reg=GB * GROUP_SIZE,
                    elem_size=BATCH,
                )
                ot = [
                    opool.tile([128, GB * BATCH], F32, name=f"ot{h}", tag=f"ot{h}")
                    for h in range(O_HALVES)
                ]
                for j in range(GB):
                    g = c * GB + j
                    for h in range(O_HALVES):
                        ps = ppool.tile([128, BATCH], F32, name="ps")
                        nc.tensor.matmul(
                            out=ps[:],
                            lhsT=w_tiles[c][:, j * OUT_DIM + h * 128 : j * OUT_DIM + (h + 1) * 128],
                            rhs=xg[:, j, :],
                            start=True,
                            stop=True,
                        )
                        dst = ot[h][:, j * BATCH : (j + 1) * BATCH]
                        bias_ap = bias_t[:, h * G_PER + g : h * G_PER + g + 1]
                        if h == 0:
                            nc.scalar.add(dst, ps[:], bias_ap)
                        else:
                            nc.vector.tensor_scalar_add(dst, ps[:], bias_ap)
                for h in range(O_HALVES):
                    nc.sync.dma_start(
                        out=out[h, :, c * GB : (c + 1) * GB, :], in_=ot[h][:]
                    )
    nc.compile()
    return nc


def build_in_maps(x, idx, W, b, gamma, beta, mean, var):
    x = np.asarray(x, dtype=np.float32)
    idx = np.asarray(idx, dtype=np.int32)
    W = np.asarray(W, dtype=np.float32)
    b = np.asarray(b, dtype=np.float32)
    gamma = np.asarray(gamma, dtype=np.float32)
    beta = np.asarray(beta, dtype=np.float32)
    mean = np.asarray(mean, dtype=np.float32)
    var = np.asarray(var, dtype=np.float32)

    # Fold BN into weights / bias (host)
    inv = (gamma / np.sqrt(var + BN_EPS)).astype(np.float32)       # [256]
    shift = (beta - mean * inv).astype(np.float32)                 # [256]
    Wf = W * inv[None, None, :]                                    # [360,128,256]
    bias = b * inv[None, :] + shift[None, :]                       # [360,256]
    xT = np.ascontiguousarray(x.T)                                 # [65536,256]

    in_maps = []
    for k in range(N_CORES):
        gs = slice(k * G_PER, (k + 1) * G_PER)
        Wk = Wf[gs]                                                # [45,128,256]
        Wd = np.ascontiguousarray(
            Wk.transpose(1, 0, 2).reshape(GROUP_SIZE, G_PER * OUT_DIM)
        )
        idx_k = idx[gs]                                            # [45,128]
        rows, inv_pos = np.unique(idx_k.ravel(), return_inverse=True)
        assert len(rows) <= N_ROWS
        xTc = np.zeros((N_ROWS, BATCH), dtype=np.float32)
        xTc[: len(rows)] = xT[rows]
        compact = inv_pos.astype(np.int16)                         # [5760] i = g*128+s
        idx16 = np.empty((128, IDX_COLS), dtype=np.int16)
        seg_len = GB * GROUP_SIZE                                  # 1152
        for c in range(N_CHUNKS):
            seg = compact[c * seg_len : (c + 1) * seg_len]
            wrap = seg.reshape(IDX_COLS_C, 16).T                   # [16, 72]
            idx16[:, c * IDX_COLS_C : (c + 1) * IDX_COLS_C] = np.tile(wrap, (8, 1))
        bk = bias[gs]                                              # [45,256]
        biasd = np.ascontiguousarray(
            bk.T.reshape(O_HALVES, 128, G_PER).transpose(1, 0, 2).reshape(
                128, O_HALVES * G_PER
            )
        )
        in_maps.append({"xTc": xTc, "Wd": Wd, "idx16": idx16, "biasd": biasd})
    return in_maps


def assemble_output(results):
    outs = []
    for k in range(N_CORES):
        o = results[k]["out"]                                      # [2,128,45,256]
        outs.append(o.transpose(3, 2, 0, 1).reshape(BATCH, G_PER, OUT_DIM))
    return np.ascontiguousarray(np.concatenate(outs, axis=1))


def kernel(x, idx, W, b, gamma, beta, mean, var):
    in_maps = build_in_maps(x, idx, W, b, gamma, beta, mean, var)

    if "nc" not in _cached:
        _cached["nc"] = build_kernel()
    nc = _cached["nc"]

    res = run_bass_kernel_spmd(nc, in_maps, core_ids=list(range(N_CORES)))
    return assemble_output(res.results)


# revision 13
# speedup vs baseline: 13.5945x; 13.5945x over previous
"""Trainium2 Bass kernel for nn_LocallyDense.

Computation (reference):
    xg[b,g,s] = x[b, idx[g,s]]                        # gather
    out[b,g,o] = sum_s xg[b,g,s] * W[g,s,o] + b[g,o]  # 360 grouped dense
    out = out * (gamma*rsqrt(var+eps)) + (beta - mean*gamma*rsqrt(var+eps))

Shapes: x [256, 65536] f32, idx [360, 128] i32, W [360,128,256] f32,
b [360,256], gamma/beta/mean/var [256].  Output [256, 360, 256] f32.

Strategy: shard the 360 groups over 8 cores (45 groups each; every core
keeps the full batch, so no collectives are needed — the host
concatenates the per-core outputs).  BN scale is folded into W on the
host, BN shift + b folded into a per-(group,out) bias.

The host transposes x to xT [65536, 256] (one voxel row = 1 KB
contiguous) and *compacts* it per core: each core only needs the <=5760
distinct voxel rows its 45 groups reference, so the host ships
xTc [5760, 256] plus remapped int16 indices.  The device gathers voxel
rows with the SWDGE `dma_gather` primitive (dst[i%128, i//128, :] =
src[idx[i], :]), which with i = g*128 + s yields exactly the transposed
activation tile xgT[s, g, b] needed for the grouped matmul.

Device per group g (o_half h in {0,1}):
    psum[128_o, 256_b] = W[g][:, h*128:+128].T @ xgT[:, g, :]  (TensorE)
    sbuf_out = psum + bias[g, h]     (ACT / DVE per-partition bias add)
    DMA out -> out_dev[h, o_local, g, b]  (layout gives k*1KB contiguous
                                           store descriptors)

Host epilogue: concatenate the 8 core outputs and transpose to [B,G,O].
"""

import numpy as np

import concourse.bass as bass
import concourse.bacc as bacc
import concourse.mybir as mybir
import concourse.tile as tile
from concourse.bass_utils import run_bass_kernel_spmd

# Problem constants (hardcoded per harness contract)
N_GROUPS, GROUP_SIZE, OUT_DIM = 360, 128, 256
N_VOXELS, BATCH = 65536, 256
BN_EPS = 1e-3
N_CORES = 8
G_PER = N_GROUPS // N_CORES        # 45 groups per core
O_HALVES = OUT_DIM // 128          # 2
N_ROWS = G_PER * GROUP_SIZE        # 5760 gathered rows per core

# Tuning knobs
# GB*128 indices per dma_gather call; with single_packet=True the per-lane
# packet is num_idxs/16+1 descriptors and must stay <= 64 -> GB <= 7.
GB = 5                             # groups per pipeline chunk
N_CHUNKS = G_PER // GB             # 9
assert N_CHUNKS * GB == G_PER
IDX_COLS = N_ROWS // 16            # 360 int16 per partition (wrap layout)
IDX_COLS_C = GB * GROUP_SIZE // 16  # 72 per chunk

F32 = mybir.dt.float32
I16 = mybir.dt.int16

_cached = {}


def build_kernel(iters: int = 1) -> bass.Bass:
    """iters>1 wraps the body in an on-device loop (used only for timing)."""
    nc = bacc.Bacc("TRN2", target_bir_lowering=False, debug=False)
    # Inputs (per core)
    xTc = nc.dram_tensor("xTc", [N_ROWS, BATCH], F32, kind="ExternalInput")
    # Wd[s, g*256+o] = W_folded[g, s, o]
    Wd = nc.dram_tensor("Wd", [GROUP_SIZE, G_PER * OUT_DIM], F32, kind="ExternalInput")
    # idx16: wrap layout per gather chunk, replicated over the 8 Q7 cores
    idx16 = nc.dram_tensor("idx16", [128, IDX_COLS], I16, kind="ExternalInput")
    # biasd[p, h*G_PER+g] = bias[g, h*128+p]
    biasd = nc.dram_tensor("biasd", [128, O_HALVES * G_PER], F32, kind="ExternalInput")
    # Output: out_dev[h, o_local, g, b] = result[b, g, h*128+o_local]
    out = nc.dram_tensor(
        "out", [O_HALVES, 128, G_PER, BATCH], F32, kind="ExternalOutput"
    )

    with tile.TileContext(nc) as tc:
        with (
            tc.tile_pool(name="const", bufs=1) as cpool,
            tc.tile_pool(name="wpool", bufs=1) as wpool,
            tc.tile_pool(name="xpool", bufs=3) as xpool,
            tc.tile_pool(name="opool", bufs=4) as opool,
            tc.tile_pool(name="ppool", bufs=8, space="PSUM") as ppool,
        ):
            idx_t = cpool.tile([128, IDX_COLS], I16, name="idx_t")
            nc.sync.dma_start(out=idx_t[:], in_=idx16[:])
            bias_t = cpool.tile([128, O_HALVES * G_PER], F32, name="bias_t")
            nc.sync.dma_start(out=bias_t[:], in_=biasd[:])

            def body():
                # Resident weight tiles, one per chunk; per-partition
                # descriptors are GB KB contiguous.
                w_tiles = []
                for c in range(N_CHUNKS):
                    w_t = wpool.tile([GROUP_SIZE, GB * OUT_DIM], F32, name=f"w_{c}")
                    nc.sync.dma_start(
                        out=w_t[:],
                        in_=Wd[:, c * GB * OUT_DIM : (c + 1) * GB * OUT_DIM],
                    )
                    w_tiles.append(w_t)

                for c in range(N_CHUNKS):
                    # Gather GB*128 voxel rows:
                    #   xg[s, j, :] = xTc[cidx[(c*GB+j)*128+s], :]
                    xg = xpool.tile([GROUP_SIZE, GB, BATCH], F32, name="xg")
                    nc.gpsimd.dma_gather(
                        out_ap=xg[:],
                        in_ap=xTc[:],
                        idxs_ap=idx_t[:, c * IDX_COLS_C : (c + 1) * IDX_COLS_C],
                        num_idxs=GB * GROUP_SIZE,
                        num_idxs_reg=GB * GROUP_SIZE,
                        elem_size=BATCH,
                    )
                    ot = [
                        opool.tile([128, GB * BATCH], F32, name=f"ot{h}", tag=f"ot{h}")
                        for h in range(O_HALVES)
                    ]
                    for j in range(GB):
                        g = c * GB + j
                        for h in range(O_HALVES):
                            ps = ppool.tile([128, BATCH], F32, name="ps")
                            nc.tensor.matmul(
                                out=ps[:],
                                lhsT=w_tiles[c][
                                    :, j * OUT_DIM + h * 128 : j * OUT_DIM + (h + 1) * 128
                                ],
                                rhs=xg[:, j, :],
                                start=True,
                                stop=True,
                            )
                            dst = ot[h][:, j * BATCH : (j + 1) * BATCH]
                            bias_ap = bias_t[:, h * G_PER + g : h * G_PER + g + 1]
                            if h == 0:
                                nc.scalar.add(dst, ps[:], bias_ap)
                            else:
                                nc.vector.tensor_scalar_add(dst, ps[:], bias_ap)
                    for h in range(O_HALVES):
                        nc.sync.dma_start(
                            out=out[h, :, c * GB : (c + 1) * GB, :], in_=ot[h][:]
                        )

            if iters == 1:
                body()
            else:
                with tc.For_i(0, iters, 1):
                    body()
    nc.compile()
    return nc


def build_in_maps(x, idx, W, b, gamma, beta, mean, var):
    x = np.asarray(x, dtype=np.float32)
    idx = np.asarray(idx, dtype=np.int32)
    W = np.asarray(W, dtype=np.float32)
    b = np.asarray(b, dtype=np.float32)
    gamma = np.asarray(gamma, dtype=np.float32)
    beta = np.asarray(beta, dtype=np.float32)
    mean = np.asarray(mean, dtype=np.float32)
    var = np.asarray(var, dtype=np.float32)

    # Fold BN into weights / bias (host)
    inv = (gamma / np.sqrt(var + BN_EPS)).astype(np.float32)       # [256]
    shift = (beta - mean * inv).astype(np.float32)                 # [256]
    Wf = W * inv[None, None, :]                                    # [360,128,256]
    bias = b * inv[None, :] + shift[None, :]                       # [360,256]
    xT = np.ascontiguousarray(x.T)                                 # [65536,256]

    in_maps = []
    for k in range(N_CORES):
        gs = slice(k * G_PER, (k + 1) * G_PER)
        Wk = Wf[gs]                                                # [45,128,256]
        Wd = np.ascontiguousarray(
            Wk.transpose(1, 0, 2).reshape(GROUP_SIZE, G_PER * OUT_DIM)
        )
        idx_k = idx[gs]                                            # [45,128]
        rows, inv_pos = np.unique(idx_k.ravel(), return_inverse=True)
        assert len(rows) <= N_ROWS
        xTc = np.zeros((N_ROWS, BATCH), dtype=np.float32)
        xTc[: len(rows)] = xT[rows]
        compact = inv_pos.astype(np.int16)                         # [5760] i = g*128+s
        idx16 = np.empty((128, IDX_COLS), dtype=np.int16)
        seg_len = GB * GROUP_SIZE                                  # 1152
        for c in range(N_CHUNKS):
            seg = compact[c * seg_len : (c + 1) * seg_len]
            wrap = seg.reshape(IDX_COLS_C, 16).T                   # [16, 72]
            idx16[:, c * IDX_COLS_C : (c + 1) * IDX_COLS_C] = np.tile(wrap, (8, 1))
        bk = bias[gs]                                              # [45,256]
        biasd = np.ascontiguousarray(
            bk.T.reshape(O_HALVES, 128, G_PER).transpose(1, 0, 2).reshape(
                128, O_HALVES * G_PER
            )
        )
        in_maps.append({"xTc": xTc, "Wd": Wd, "idx16": idx16, "biasd": biasd})
    return in_maps


def assemble_output(results):
    outs = []
    for k in range(N_CORES):
        o = results[k]["out"]                                      # [2,128,45,256]
        outs.append(o.transpose(3, 2, 0, 1).reshape(BATCH, G_PER, OUT_DIM))
    return np.ascontiguousarray(np.concatenate(outs, axis=1))


def kernel(x, idx, W, b, gamma, beta, mean, var):
    in_maps = build_in_maps(x, idx, W, b, gamma, beta, mean, var)

    if "nc" not in _cached:
        _cached["nc"] = build_kernel()
    nc = _cached["nc"]

    res = run_bass_kernel_spmd(nc, in_maps, core_ids=list(range(N_CORES)))
    return assemble_output(res.results)


# revision 16
# speedup vs baseline: 40.9783x; 3.0143x over previous
"""Trainium2 Bass kernel for nn_LocallyDense.

Computation (reference):
    xg[b,g,s] = x[b, idx[g,s]]                        # gather
    out[b,g,o] = sum_s xg[b,g,s] * W[g,s,o] + b[g,o]  # 360 grouped dense
    out = out * (gamma*rsqrt(var+eps)) + (beta - mean*gamma*rsqrt(var+eps))

Shapes: x [256, 65536] f32, idx [360, 128] i32, W [360,128,256] f32,
b [360,256], gamma/beta/mean/var [256].  Output [256, 360, 256] f32.

Strategy: shard the 360 groups over 8 cores (45 groups each; every core
keeps the full batch, so no collectives are needed — the host
concatenates the per-core outputs).  BN scale is folded into W on the
host, BN shift + b folded into a per-(group,out) bias.

The host transposes x to xT [65536, 256] (one voxel row = 1 KB
contiguous) and *compacts* it per core: each core only needs the <=5760
distinct voxel rows its 45 groups reference, so the host ships
xTc [5760, 256] plus remapped int16 indices.  The device gathers voxel
rows with the SWDGE `dma_gather` primitive (dst[i%128, i//128, :] =
src[idx[i], :]), which with i = g*128 + s yields exactly the transposed
activation tile xgT[s, g, b] needed for the grouped matmul.

Device per group g (o_half h in {0,1}):
    psum[128_o, 256_b] = W[g][:, h*128:+128].T @ xgT[:, g, :]  (TensorE)
    sbuf_out = psum + bias[g, h]     (ACT / DVE per-partition bias add)
    DMA out -> out_dev[h, o_local, g, b]  (layout gives k*1KB contiguous
                                           store descriptors)

Host epilogue: concatenate the 8 core outputs and transpose to [B,G,O].
"""

import numpy as np

import concourse.bass as bass
import concourse.bacc as bacc
import concourse.mybir as mybir
import concourse.tile as tile
from concourse.bass_utils import run_bass_kernel_spmd

# Problem constants (hardcoded per harness contract)
N_GROUPS, GROUP_SIZE, OUT_DIM = 360, 128, 256
N_VOXELS, BATCH = 65536, 256
BN_EPS = 1e-3
N_CORES = 8
G_PER = N_GROUPS // N_CORES        # 45 groups per core
O_HALVES = OUT_DIM // 128          # 2
N_ROWS = G_PER * GROUP_SIZE        # 5760 gathered rows per core

# Tuning knobs
# GB*128 indices per dma_gather call; with single_packet=True the per-lane
# packet is num_idxs/16+1 descriptors and must stay <= 64 -> GB <= 7.
GB = 5                             # groups per pipeline chunk
N_CHUNKS = G_PER // GB             # 9
assert N_CHUNKS * GB == G_PER
# Gather call granularity: GGB groups per dma_gather (multiple of GB so the
# consuming chunks line up); single-packet only legal when GGB <= 7.
GGB = GB                           # groups per gather call
N_GCHUNKS = G_PER // GGB
assert N_GCHUNKS * GGB == G_PER and GGB % GB == 0
SINGLE_PACKET = GGB * GROUP_SIZE // 16 + 1 <= 64
GATHER_QUEUES = 1                  # SWDGE queue fan-out for dma_gather
IDX_COLS = N_ROWS // 16            # 360 int16 per partition (wrap layout)
IDX_COLS_C = GGB * GROUP_SIZE // 16  # per gather call

F32 = mybir.dt.float32
I16 = mybir.dt.int16

_cached = {}


def build_kernel(iters: int = 1, skip: frozenset = frozenset()) -> bass.Bass:
    """iters>1 wraps the body in an on-device loop (used only for timing).
    skip: ablation flags for benchmarking ("gather", "mm", "store", "wload")."""
    nc = bacc.Bacc("TRN2", target_bir_lowering=False, debug=False)
    # Inputs (per core)
    xTc = nc.dram_tensor("xTc", [N_ROWS, BATCH], F32, kind="ExternalInput")
    # Wd[s, g*256+o] = W_folded[g, s, o]
    Wd = nc.dram_tensor("Wd", [GROUP_SIZE, G_PER * OUT_DIM], F32, kind="ExternalInput")
    # idx16: wrap layout per gather chunk, replicated over the 8 Q7 cores
    idx16 = nc.dram_tensor("idx16", [128, IDX_COLS], I16, kind="ExternalInput")
    # biasd[p, h*G_PER+g] = bias[g, h*128+p]
    biasd = nc.dram_tensor("biasd", [128, O_HALVES * G_PER], F32, kind="ExternalInput")
    # Output: out_dev[h, o_local, g, b] = result[b, g, h*128+o_local]
    out = nc.dram_tensor(
        "out", [O_HALVES, 128, G_PER, BATCH], F32, kind="ExternalOutput"
    )

    with tile.TileContext(nc) as tc:
        with (
            tc.tile_pool(name="const", bufs=1) as cpool,
            tc.tile_pool(name="wpool", bufs=1) as wpool,
            tc.tile_pool(name="xpool", bufs=3) as xpool,
            tc.tile_pool(name="opool", bufs=4) as opool,
            tc.tile_pool(name="ppool", bufs=8, space="PSUM") as ppool,
        ):
            idx_t = cpool.tile([128, IDX_COLS], I16, name="idx_t")
            nc.sync.dma_start(out=idx_t[:], in_=idx16[:])
            bias_t = cpool.tile([128, O_HALVES * G_PER], F32, name="bias_t")
            nc.sync.dma_start(out=bias_t[:], in_=biasd[:])

            def body():
                # Resident weight tiles, one per chunk; per-partition
                # descriptors are GB KB contiguous.
                w_tiles = []
                for c in range(N_CHUNKS):
                    w_t = wpool.tile([GROUP_SIZE, GB * OUT_DIM], F32, name=f"w_{c}")
                    if "wload" not in skip:
                        nc.sync.dma_start(
                            out=w_t[:],
                            in_=Wd[:, c * GB * OUT_DIM : (c + 1) * GB * OUT_DIM],
                        )
                    w_tiles.append(w_t)

                for c in range(N_CHUNKS):
                    # Gather GB*128 voxel rows:
                    #   xg[s, j, :] = xTc[cidx[(c*GB+j)*128+s], :]
                    xg = xpool.tile([GROUP_SIZE, GB, BATCH], F32, name="xg")
                    if "gather" not in skip:
                        nc.gpsimd.dma_gather(
                            out_ap=xg[:],
                            in_ap=xTc[:],
                            idxs_ap=idx_t[:, c * IDX_COLS_C : (c + 1) * IDX_COLS_C],
                            num_idxs=GB * GROUP_SIZE,
                            num_idxs_reg=GB * GROUP_SIZE,
                            elem_size=BATCH,
                        )
                    ot = [
                        opool.tile([128, GB * BATCH], F32, name=f"ot{h}", tag=f"ot{h}")
                        for h in range(O_HALVES)
                    ]
                    if "mm" not in skip:
                        for j in range(GB):
                            g = c * GB + j
                            for h in range(O_HALVES):
                                ps = ppool.tile([128, BATCH], F32, name="ps")
                                nc.tensor.matmul(
                                    out=ps[:],
                                    lhsT=w_tiles[c][
                                        :, j * OUT_DIM + h * 128 : j * OUT_DIM + (h + 1) * 128
                                    ],
                                    rhs=xg[:, j, :],
                                    start=True,
                                    stop=True,
                                )
                                dst = ot[h][:, j * BATCH : (j + 1) * BATCH]
                                bias_ap = bias_t[:, h * G_PER + g : h * G_PER + g + 1]
                                if h == 0:
                                    nc.scalar.add(dst, ps[:], bias_ap)
                                else:
                                    nc.vector.tensor_scalar_add(dst, ps[:], bias_ap)
                    if "store" not in skip:
                        for h in range(O_HALVES):
                            nc.sync.dma_start(
                                out=out[h, :, c * GB : (c + 1) * GB, :], in_=ot[h][:]
                            )

            if iters == 1:
                body()
            else:
                with tc.For_i(0, iters, 1):
                    body()
    nc.compile()
    return nc


def build_in_maps(x, idx, W, b, gamma, beta, mean, var):
    x = np.asarray(x, dtype=np.float32)
    idx = np.asarray(idx, dtype=np.int32)
    W = np.asarray(W, dtype=np.float32)
    b = np.asarray(b, dtype=np.float32)
    gamma = np.asarray(gamma, dtype=np.float32)
    beta = np.asarray(beta, dtype=np.float32)
    mean = np.asarray(mean, dtype=np.float32)
    var = np.asarray(var, dtype=np.float32)

    # Fold BN into weights / bias (host)
    inv = (gamma / np.sqrt(var + BN_EPS)).astype(np.float32)       # [256]
    shift = (beta - mean * inv).astype(np.float32)                 # [256]
    Wf = W * inv[None, None, :]                                    # [360,128,256]
    bias = b * inv[None, :] + shift[None, :]                       # [360,256]
    xT = np.ascontiguousarray(x.T)                                 # [65536,256]

    in_maps = []
    for k in range(N_CORES):
        gs = slice(k * G_PER, (k + 1) * G_PER)
        Wk = Wf[gs]                                                # [45,128,256]
        Wd = np.ascontiguousarray(
            Wk.transpose(1, 0, 2).reshape(GROUP_SIZE, G_PER * OUT_DIM)
        )
        idx_k = idx[gs]                                            # [45,128]
        rows, inv_pos = np.unique(idx_k.ravel(), return_inverse=True)
        assert len(rows) <= N_ROWS
        xTc = np.zeros((N_ROWS, BATCH), dtype=np.float32)
        xTc[: len(rows)] = xT[rows]
        compact = inv_pos.astype(np.int16)                         # [5760] i = g*128+s
        idx16 = np.empty((128, IDX_COLS), dtype=np.int16)
        seg_len = GB * GROUP_SIZE                                  # 1152
        for c in range(N_CHUNKS):
            seg = compact[c * seg_len : (c + 1) * seg_len]
            wrap = seg.reshape(IDX_COLS_C, 16).T                   # [16, 72]
            idx16[:, c * IDX_COLS_C : (c + 1) * IDX_COLS_C] = np.tile(wrap, (8, 1))
        bk = bias[gs]                                              # [45,256]
        biasd = np.ascontiguousarray(
            bk.T.reshape(O_HALVES, 128, G_PER).transpose(1, 0, 2).reshape(
                128, O_HALVES * G_PER
            )
        )
        in_maps.append({"xTc": xTc, "Wd": Wd, "idx16": idx16, "biasd": biasd})
    return in_maps


def assemble_output(results):
    outs = []
    for k in range(N_CORES):
        o = results[k]["out"]                                      # [2,128,45,256]
        outs.append(o.transpose(3, 2, 0, 1).reshape(BATCH, G_PER, OUT_DIM))
    return np.ascontiguousarray(np.concatenate(outs, axis=1))


def kernel(x, idx, W, b, gamma, beta, mean, var):
    in_maps = build_in_maps(x, idx, W, b, gamma, beta, mean, var)

    if "nc" not in _cached:
        _cached["nc"] = build_kernel()
    nc = _cached["nc"]

    res = run_bass_kernel_spmd(nc, in_maps, core_ids=list(range(N_CORES)))
    return assemble_output(res.results)


# revision 18
# speedup vs baseline: 44.5027x; 1.0860x over previous
"""Trainium2 Bass kernel for nn_LocallyDense.

Computation (reference):
    xg[b,g,s] = x[b, idx[g,s]]                        # gather
    out[b,g,o] = sum_s xg[b,g,s] * W[g,s,o] + b[g,o]  # 360 grouped dense
    out = out * (gamma*rsqrt(var+eps)) + (beta - mean*gamma*rsqrt(var+eps))

Shapes: x [256, 65536] f32, idx [360, 128] i32, W [360,128,256] f32,
b [360,256], gamma/beta/mean/var [256].  Output [256, 360, 256] f32.

Strategy: shard the 360 groups over 8 cores (45 groups each; every core
keeps the full batch, so no collectives are needed — the host
concatenates the per-core outputs).  BN scale is folded into W on the
host, BN shift + b folded into a per-(group,out) bias.

The host transposes x to xT [65536, 256] (one voxel row = 1 KB
contiguous) and *compacts* it per core: each core only needs the <=5760
distinct voxel rows its 45 groups reference, so the host ships
xTc [5760, 256] plus remapped int16 indices.  The device gathers voxel
rows with the SWDGE `dma_gather` primitive (dst[i%128, i//128, :] =
src[idx[i], :]), which with i = g*128 + s yields exactly the transposed
activation tile xgT[s, g, b] needed for the grouped matmul.

Device per group g (o_half h in {0,1}):
    psum[128_o, 256_b] = W[g][:, h*128:+128].T @ xgT[:, g, :]  (TensorE)
    sbuf_out = psum + bias[g, h]     (ACT / DVE per-partition bias add)
    DMA out -> out_dev[h, o_local, g, b]  (layout gives k*1KB contiguous
                                           store descriptors)

Host epilogue: concatenate the 8 core outputs and transpose to [B,G,O].
"""

import numpy as np

import concourse.bass as bass
import concourse.bacc as bacc
import concourse.mybir as mybir
import concourse.tile as tile
from concourse.bass_utils import run_bass_kernel_spmd

# Problem constants (hardcoded per harness contract)
N_GROUPS, GROUP_SIZE, OUT_DIM = 360, 128, 256
N_VOXELS, BATCH = 65536, 256
BN_EPS = 1e-3
N_CORES = 8
G_PER = N_GROUPS // N_CORES        # 45 groups per core
O_HALVES = OUT_DIM // 128          # 2
N_ROWS = G_PER * GROUP_SIZE        # 5760 gathered rows per core

# Tuning knobs
# GB*128 indices per dma_gather call; with single_packet=True the per-lane
# packet is num_idxs/16+1 descriptors and must stay <= 64 -> GB <= 7.
GB = 5                             # groups per pipeline chunk
N_CHUNKS = G_PER // GB             # 9
assert N_CHUNKS * GB == G_PER
# Gather call granularity: GGB groups per dma_gather (multiple of GB so the
# consuming chunks line up); single-packet only legal when GGB <= 7.
GGB = GB                           # groups per gather call
N_GCHUNKS = G_PER // GGB
assert N_GCHUNKS * GGB == G_PER and GGB % GB == 0
SINGLE_PACKET = GGB * GROUP_SIZE // 16 + 1 <= 64
GATHER_QUEUES = 1                  # SWDGE queue fan-out for dma_gather
IDX_COLS = N_ROWS // 16            # 360 int16 per partition (wrap layout)
IDX_COLS_C = GGB * GROUP_SIZE // 16  # per gather call

F32 = mybir.dt.float32
I16 = mybir.dt.int16

_cached = {}


def build_kernel(iters: int = 1, skip: frozenset = frozenset()) -> bass.Bass:
    """iters>1 wraps the body in an on-device loop (used only for timing).
    skip: ablation flags for benchmarking ("gather", "mm", "store", "wload")."""
    nc = bacc.Bacc("TRN2", target_bir_lowering=False, debug=False)
    # Inputs (per core)
    xTc = nc.dram_tensor("xTc", [N_ROWS, BATCH], F32, kind="ExternalInput")
    # Wd[s, g*256+o] = W_folded[g, s, o]
    Wd = nc.dram_tensor("Wd", [GROUP_SIZE, G_PER * OUT_DIM], F32, kind="ExternalInput")
    # idx16: wrap layout per gather chunk, replicated over the 8 Q7 cores
    idx16 = nc.dram_tensor("idx16", [128, IDX_COLS], I16, kind="ExternalInput")
    # biasd[p, h*G_PER+g] = bias[g, h*128+p]
    biasd = nc.dram_tensor("biasd", [128, O_HALVES * G_PER], F32, kind="ExternalInput")
    # Output: out_dev[h, o_local, g, b] = result[b, g, h*128+o_local]
    out = nc.dram_tensor(
        "out", [O_HALVES, 128, G_PER, BATCH], F32, kind="ExternalOutput"
    )

    with tile.TileContext(nc) as tc:
        with (
            tc.tile_pool(name="const", bufs=1) as cpool,
            tc.tile_pool(name="wpool", bufs=1) as wpool,
            tc.tile_pool(name="xpool", bufs=3) as xpool,
            tc.tile_pool(name="opool", bufs=4) as opool,
            tc.tile_pool(name="ppool", bufs=8, space="PSUM") as ppool,
        ):
            idx_t = cpool.tile([128, IDX_COLS], I16, name="idx_t")
            nc.sync.dma_start(out=idx_t[:], in_=idx16[:])
            bias_t = cpool.tile([128, O_HALVES * G_PER], F32, name="bias_t")
            nc.sync.dma_start(out=bias_t[:], in_=biasd[:])

            def load_w():
                # Resident weight tiles, one per chunk; per-partition
                # descriptors are GB KB contiguous.
                w_tiles = []
                for c in range(N_CHUNKS):
                    w_t = wpool.tile([GROUP_SIZE, GB * OUT_DIM], F32, name=f"w_{c}")
                    nc.sync.dma_start(
                        out=w_t[:],
                        in_=Wd[:, c * GB * OUT_DIM : (c + 1) * GB * OUT_DIM],
                    )
                    w_tiles.append(w_t)
                return w_tiles

            def do_gather(gc):
                # Gather GGB*128 voxel rows:
                #   xg[s, j, :] = xTc[cidx[(gc*GGB+j)*128+s], :]
                xg = xpool.tile([GROUP_SIZE, GGB, BATCH], F32, name="xg")
                nc.gpsimd.dma_gather(
                    out_ap=xg[:],
                    in_ap=xTc[:],
                    idxs_ap=idx_t[:, gc * IDX_COLS_C : (gc + 1) * IDX_COLS_C],
                    num_idxs=GGB * GROUP_SIZE,
                    num_idxs_reg=GGB * GROUP_SIZE,
                    elem_size=BATCH,
                    single_packet=SINGLE_PACKET,
                    queue_num=gc % GATHER_QUEUES,
                )
                return xg

            # Bench-ablation hoisting: if a producer is skipped inside the
            # loop but its consumer (mm) remains, produce the tiles once up
            # front so every tile read inside the loop has a writer.
            w_static = load_w() if ("wload" in skip and "mm" not in skip) else None
            xg_static = (
                [do_gather(gc) for gc in range(N_GCHUNKS)]
                if ("gather" in skip and "mm" not in skip)
                else None
            )

            def body():
                if "wload" not in skip:
                    w_tiles = load_w()
                else:
                    w_tiles = w_static
                xg_tiles = xg_static
                if "gather" not in skip:
                    xg_tiles = [do_gather(gc) for gc in range(N_GCHUNKS)]
                for c in range(N_CHUNKS):
                    ot = [
                        opool.tile([128, GB * BATCH], F32, name=f"ot{h}", tag=f"ot{h}")
                        for h in range(O_HALVES)
                    ]
                    if "mm" not in skip:
                        gc, sub = divmod(c, GGB // GB)
                        xg = xg_tiles[gc]
                        for j in range(GB):
                            g = c * GB + j
                            for h in range(O_HALVES):
                                ps = ppool.tile([128, BATCH], F32, name="ps")
                                nc.tensor.matmul(
                                    out=ps[:],
                                    lhsT=w_tiles[c][
                                        :, j * OUT_DIM + h * 128 : j * OUT_DIM + (h + 1) * 128
                                    ],
                                    rhs=xg[:, sub * GB + j, :],
                                    start=True,
                                    stop=True,
                                )
                                dst = ot[h][:, j * BATCH : (j + 1) * BATCH]
                                bias_ap = bias_t[:, h * G_PER + g : h * G_PER + g + 1]
                                if h == 0:
                                    nc.scalar.add(dst, ps[:], bias_ap)
                                else:
                                    nc.vector.tensor_scalar_add(dst, ps[:], bias_ap)
                    if "store" not in skip:
                        for h in range(O_HALVES):
                            nc.sync.dma_start(
                                out=out[h, :, c * GB : (c + 1) * GB, :], in_=ot[h][:]
                            )

            if iters == 1:
                body()
            else:
                with tc.For_i(0, iters, 1):
                    body()
    nc.compile()
    return nc


def build_in_maps(x, idx, W, b, gamma, beta, mean, var):
    x = np.asarray(x, dtype=np.float32)
    idx = np.asarray(idx, dtype=np.int32)
    W = np.asarray(W, dtype=np.float32)
    b = np.asarray(b, dtype=np.float32)
    gamma = np.asarray(gamma, dtype=np.float32)
    beta = np.asarray(beta, dtype=np.float32)
    mean = np.asarray(mean, dtype=np.float32)
    var = np.asarray(var, dtype=np.float32)

    # Fold BN into weights / bias (host)
    inv = (gamma / np.sqrt(var + BN_EPS)).astype(np.float32)       # [256]
    shift = (beta - mean * inv).astype(np.float32)                 # [256]
    Wf = W * inv[None, None, :]                                    # [360,128,256]
    bias = b * inv[None, :] + shift[None, :]                       # [360,256]
    xT = np.ascontiguousarray(x.T)                                 # [65536,256]

    in_maps = []
    for k in range(N_CORES):
        gs = slice(k * G_PER, (k + 1) * G_PER)
        Wk = Wf[gs]                                                # [45,128,256]
        Wd = np.ascontiguousarray(
            Wk.transpose(1, 0, 2).reshape(GROUP_SIZE, G_PER * OUT_DIM)
        )
        idx_k = idx[gs]                                            # [45,128]
        rows, inv_pos = np.unique(idx_k.ravel(), return_inverse=True)
        assert len(rows) <= N_ROWS
        xTc = np.zeros((N_ROWS, BATCH), dtype=np.float32)
        xTc[: len(rows)] = xT[rows]
        compact = inv_pos.astype(np.int16)                         # [5760] i = g*128+s
        idx16 = np.empty((128, IDX_COLS), dtype=np.int16)
        seg_len = GGB * GROUP_SIZE
        for c in range(N_GCHUNKS):
            seg = compact[c * seg_len : (c + 1) * seg_len]
            wrap = seg.reshape(IDX_COLS_C, 16).T
            idx16[:, c * IDX_COLS_C : (c + 1) * IDX_COLS_C] = np.tile(wrap, (8, 1))
        bk = bias[gs]                                              # [45,256]
        biasd = np.ascontiguousarray(
            bk.T.reshape(O_HALVES, 128, G_PER).transpose(1, 0, 2).reshape(
                128, O_HALVES * G_PER
            )
        )
        in_maps.append({"xTc": xTc, "Wd": Wd, "idx16": idx16, "biasd": biasd})
    return in_maps


def assemble_output(results):
    outs = []
    for k in range(N_CORES):
        o = results[k]["out"]                                      # [2,128,45,256]
        outs.append(o.transpose(3, 2, 0, 1).reshape(BATCH, G_PER, OUT_DIM))
    return np.ascontiguousarray(np.concatenate(outs, axis=1))


def kernel(x, idx, W, b, gamma, beta, mean, var):
    in_maps = build_in_maps(x, idx, W, b, gamma, beta, mean, var)

    if "nc" not in _cached:
        _cached["nc"] = build_kernel()
    nc = _cached["nc"]

    res = run_bass_kernel_spmd(nc, in_maps, core_ids=list(range(N_CORES)))
    return assemble_output(res.results)
